# revision 6
# baseline (speedup 1.0000x reference)
"""DLRM (26-table embedding + pairwise interaction + MLPs) on 8 Trainium2
NeuronCores, data-parallel over the batch (each core owns B/8 = 2048 samples
and a full replica of the embedding tables; no collectives needed).

v2 — pipelined around the SWDGE gather stream (the hard floor: ~3ns/descriptor
of Q7 emission, 53248 descriptors/core ~ 165us). All compute hides under it:

  1. bottom MLP 13->512->256->64 (lhsT=W matmuls), overlaps first gathers
  2. gathers chunked into 4 sample-groups x 26 tables (512 idx each) on 4
     SWDGE queues with a deep buffer pool, so descriptor emission never
     stalls on drains and the 4 queues drain concurrently
  3. per group: DVE predicated-merge (row-in-unit select), HWDGE
     DMA-transposes into feature-major slabs tmt[64h+d, rI, j] where rI is an
     interleaved feature order (feature 4g+ii at row 7ii+g) chosen so every
     later access pattern is a plain nested-stride walk
  4. grams: 2 samples per matmul via block-diagonal zero-padded rhs
     rz[128, j, 64] (cols 32h+r), lhsT = dense slab column; out [27, 64]
     holds Z of both pair samples side by side
  5. Z relayout (DVE, PSUM->SBUF) into K-stacked Zst[27*ii + r, g, s] so the
     interaction fold runs as 7 K=108 matmuls per output block instead of
     27 K=27 ones; W2 is symmetrized/half-weighted/zero-diag + row-permuted
     on the host to match
  6. top MLP 415->512->256->1 + sigmoid per 512-sample block, f32 output
"""

import sys

if "/opt/trn_rl_repo" not in sys.path:
    sys.path.insert(0, "/opt/trn_rl_repo")

import ml_dtypes
import numpy as np

import concourse.bass as bass
import concourse.mybir as mybir
import concourse.tile as tile
from concourse import bacc
from concourse.bass_utils import run_bass_kernel_spmd

F32 = mybir.dt.float32
BF16 = mybir.dt.bfloat16
I16 = mybir.dt.int16
U8 = mybir.dt.uint8
AF = mybir.ActivationFunctionType

N_CORES = 8
B_TOTAL = 16384
B = B_TOTAL // N_CORES  # 2048 samples per core
T = 26                  # embedding tables
V = 100000              # vocab per table
D = 64                  # embedding dim
NI = 27                 # interaction vectors per sample (26 tables + dense)
N_DENSE = 13
H1, H2 = 512, 256       # bottom MLP dims
T1, T2 = 512, 256       # top MLP dims
NU = V // 4             # 25000 gather units (512B = 4 vocab rows) per table
NG = 4                  # gather sample-groups (512 samples each)
NQ = 4                  # SWDGE queues


def _rpos(t):
    """feature t -> interleaved slab row 7*(t%4) + t//4 (t = 4g+ii -> 7ii+g)."""
    return 7 * (t % 4) + t // 4


def _build_nc():
    nc = bacc.Bacc(None, target_bir_lowering=False, num_swdge_queues=NQ)

    embg = nc.dram_tensor("embg", [T, NU, 256], BF16, kind="ExternalInput")
    gidx = nc.dram_tensor("gidx", [T, 128, B // 16], I16, kind="ExternalInput")
    gmask = nc.dram_tensor("gmask", [128, T, 3, 16], U8, kind="ExternalInput")
    dense_t = nc.dram_tensor("dense_t", [N_DENSE, B], BF16, kind="ExternalInput")
    wb0 = nc.dram_tensor("wb0", [N_DENSE, H1], BF16, kind="ExternalInput")
    bb0 = nc.dram_tensor("bb0", [128, H1 // 128], F32, kind="ExternalInput")
    wb1 = nc.dram_tensor("wb1", [H1, H2], BF16, kind="ExternalInput")
    bb1 = nc.dram_tensor("bb1", [128, H2 // 128], F32, kind="ExternalInput")
    wb2 = nc.dram_tensor("wb2", [H2, D], BF16, kind="ExternalInput")
    bb2 = nc.dram_tensor("bb2", [128, 1], F32, kind="ExternalInput")
    wt0d = nc.dram_tensor("wt0d", [D, T1], BF16, kind="ExternalInput")
    w2st = nc.dram_tensor("w2st", [128, 7 * T1], BF16, kind="ExternalInput")
    bt0 = nc.dram_tensor("bt0", [128, T1 // 128], F32, kind="ExternalInput")
    wt1 = nc.dram_tensor("wt1", [T1, T2], BF16, kind="ExternalInput")
    bt1 = nc.dram_tensor("bt1", [128, T2 // 128], F32, kind="ExternalInput")
    wt2 = nc.dram_tensor("wt2", [T2, 1], BF16, kind="ExternalInput")
    bt2 = nc.dram_tensor("bt2", [1, 1], F32, kind="ExternalInput")
    out = nc.dram_tensor("out", [1, B], F32, kind="ExternalOutput")

    with tile.TileContext(nc) as tc:
        with (
            tc.tile_pool(name="const", bufs=1) as cp,
            tc.tile_pool(name="mm", bufs=2, space="PSUM") as mmp,
            tc.tile_pool(name="gram", bufs=2, space="PSUM") as gramp,
            tc.tile_pool(name="fold", bufs=2, space="PSUM") as foldp,
            tc.tile_pool(name="tm", bufs=3) as tmp,
            tc.tile_pool(name="big", bufs=1) as bigp,
            tc.tile_pool(name="gth", bufs=10) as gthp,
            tc.tile_pool(name="fin", bufs=2) as finp,
        ):
            # ---- constants / weights ----
            wb0s = cp.tile([N_DENSE, H1], BF16)
            nc.sync.dma_start(wb0s[:], wb0[:])
            wb1s = cp.tile([128, 4, H2], BF16)
            for k in range(4):
                nc.sync.dma_start(wb1s[:, k, :], wb1[128 * k:128 * (k + 1), :])
            wb2s = cp.tile([128, 2, D], BF16)
            for k in range(2):
                nc.sync.dma_start(wb2s[:, k, :], wb2[128 * k:128 * (k + 1), :])
            wt0ds = cp.tile([D, T1], BF16)
            nc.sync.dma_start(wt0ds[:], wt0d[:])
            w2s = cp.tile([128, 7, T1], BF16)
            nc.sync.dma_start(w2s[:], w2st[:])
            wt1s = cp.tile([128, 4, T2], BF16)
            for k in range(4):
                nc.sync.dma_start(wt1s[:, k, :], wt1[128 * k:128 * (k + 1), :])
            wt2s = cp.tile([128, 2, 1], BF16)
            for k in range(2):
                nc.sync.dma_start(wt2s[:, k, :], wt2[128 * k:128 * (k + 1), :])
            bb0s = cp.tile([128, H1 // 128], F32)
            nc.sync.dma_start(bb0s[:], bb0[:])
            bb1s = cp.tile([128, H2 // 128], F32)
            nc.sync.dma_start(bb1s[:], bb1[:])
            bb2s = cp.tile([128, 1], F32)
            nc.sync.dma_start(bb2s[:], bb2[:])
            bt0s = cp.tile([128, T1 // 128], F32)
            nc.sync.dma_start(bt0s[:], bt0[:])
            bt1s = cp.tile([128, T2 // 128], F32)
            nc.sync.dma_start(bt1s[:], bt1[:])
            bt2s = cp.tile([1, 1], F32)
            nc.sync.dma_start(bt2s[:], bt2[:])
            dts = cp.tile([N_DENSE, B], BF16)
            nc.sync.dma_start(dts[:], dense_t[:])
            masks = cp.tile([128, T, 3, 16], U8)
            nc.sync.dma_start(masks[:], gmask[:])
            idxs = cp.tile([128, T, B // 16], I16)
            for t in range(T):
                nc.sync.dma_start(idxs[:, t, :], gidx[t, :, :])

            # d^T replicated on both partition halves: D2[64h + d, s]
            D2 = cp.tile([128, B], BF16)

            # zero-padded block-diagonal gram rhs, ping-pong over pbt%2:
            # rz[64h'+d, buf, j, 32h + r] = slab value if h==h' else 0
            rz = bigp.tile([128, 2, 128, 64], BF16, tag="rz")
            nc.vector.memset(rz[:], 0)

            # K-stacked interaction matrix for the fold (ii-groups padded
            # to 32 partitions for the 32-aligned start rule; pad rows stay 0
            # and multiply zero w2st rows):
            # Zst[32*ii + r, g, s] = Z_s[f(r), 4g+ii]  (f = interleave inverse)
            Zst = bigp.tile([128, 7, B], BF16, tag="Zst")
            nc.vector.memset(Zst[:], 0)

            y1 = bigp.tile([128, 4, B], BF16, tag="y1")
            y2 = bigp.tile([128, 2, B], BF16, tag="y2")
            outs = bigp.tile([1, B], F32, tag="outs")

            # ---- phase 1: bottom MLP (overlaps the first gathers) ----
            for n in range(4):
                sl = slice(512 * n, 512 * (n + 1))
                h1t = bigp.tile([128, 4, 512], BF16, tag=f"h1_{n % 2}")
                for m in range(4):
                    ps = mmp.tile([128, 512], F32)
                    nc.tensor.matmul(ps[:], wb0s[:, 128 * m:128 * (m + 1)],
                                     dts[:, sl], start=True, stop=True)
                    nc.scalar.activation(h1t[:, m, :], ps[:], AF.Relu,
                                         bias=bb0s[:, m:m + 1])
                h2t = bigp.tile([128, 2, 512], BF16, tag=f"h2_{n % 2}")
                for m in range(2):
                    ps = mmp.tile([128, 512], F32)
                    for k in range(4):
                        nc.tensor.matmul(ps[:], wb1s[:, k, 128 * m:128 * (m + 1)],
                                         h1t[:, k, :], start=(k == 0), stop=(k == 3))
                    nc.scalar.activation(h2t[:, m, :], ps[:], AF.Relu,
                                         bias=bb1s[:, m:m + 1])
                ps = mmp.tile([128, 512], F32)
                for half in range(2):   # write d to both partition halves
                    for k in range(2):
                        nc.tensor.matmul(ps[64 * half:64 * half + 64, :],
                                         wb2s[:, k, :], h2t[:, k, :],
                                         start=(k == 0), stop=(k == 1),
                                         tile_position=(0, 64 * half))
                nc.scalar.activation(D2[:, sl], ps[:], AF.Relu, bias=bb2s[:])

            # ---- phases 2-6, pipelined by 512-sample group ----
            for g in range(NG):
                # -- gather + merge this group's 512 samples for all tables --
                fins = []
                for t in range(T):
                    gt = gthp.tile([128, 4, 256], BF16, tag="gt")
                    nc.gpsimd.dma_gather(gt[:], embg[t, :, :],
                                         idxs[:, t, 32 * g:32 * (g + 1)],
                                         512, 512, 256, single_packet=False,
                                         queue_num=(g * T + t) % NQ)
                    fin = finp.tile([128, 4, D], BF16, tag=f"fin{t}")
                    fins.append(fin)
                    for r in (1, 2, 3):
                        nc.vector.copy_predicated(
                            gt[:, :, 0:D],
                            masks[:, t, r - 1, 4 * g:4 * (g + 1)]
                            .to_broadcast([128, 4, D]),
                            gt[:, :, D * r:D * (r + 1)])
                    nc.vector.tensor_copy(fin[:], gt[:, :, 0:D])

                for pbt in (2 * g, 2 * g + 1):
                    buf = pbt % 2
                    lb = pbt - 2 * g
                    # -- feature-major slab via HWDGE transposes --
                    tmt = tmp.tile([128, 28, 128], BF16, tag="tmt")
                    for t in range(T):
                        eng = nc.sync if t % 2 == 0 else nc.scalar
                        src = fins[t][:].rearrange("p b d -> p (b d)")
                        eng.dma_start_transpose(
                            tmt[:, _rpos(t), :],
                            src[:, 128 * lb:128 * lb + 128])
                    for h in range(2):
                        s0 = 256 * pbt + 128 * h
                        nc.vector.tensor_copy(
                            tmt[64 * h:64 * h + 64, _rpos(26), :],
                            D2[64 * h:64 * h + 64, s0:s0 + 128])

                    # -- zero-padded gram rhs (pad cols stay 0 from memset) --
                    for h in range(2):
                        nc.vector.tensor_copy(
                            rz[64 * h:64 * h + 64, buf, :, 32 * h:32 * h + 27],
                            tmt[64 * h:64 * h + 64, 0:27, :]
                            .rearrange("p t j -> p j t"))

                    # -- grams: 2 samples per matmul, j-pair (s, s+128) --
                    pss = []
                    for half in range(2):
                        ps = gramp.tile([128, 16, 64], F32)
                        pss.append(ps)
                        for j in range(64 * half, 64 * half + 64):
                            c = j % 4
                            nc.tensor.matmul(
                                ps[32 * c:32 * c + 27, (j // 4) % 16, :],
                                tmt[:, 0:27, j], rz[:, buf, j, :],
                                start=True, stop=True,
                                tile_position=(0, 32 * c))
                    # -- Z relayout PSUM -> Zst (via Z symmetry) --
                    Zr = Zst[:].rearrange(
                        "p g (pb h sl c) -> p pb c sl h g",
                        pb=8, h=2, sl=32, c=4)
                    for half in range(2):
                        psr = pss[half][:].rearrange(
                            "p s (h x) -> p s h x", h=2)
                        for c in range(4):
                            for ii in range(4):
                                nc.vector.tensor_copy(
                                    Zr[32 * ii:32 * ii + 27, pbt, c,
                                       16 * half:16 * half + 16, :, :],
                                    psr[32 * c:32 * c + 27, :, :,
                                        7 * ii:7 * ii + 7])

                # -- fold + top-MLP layer 1 for this 512-sample block --
                sl = slice(512 * g, 512 * (g + 1))
                for m in range(4):
                    yp = foldp.tile([128, 512], F32)
                    nc.tensor.matmul(yp[:], wt0ds[:, 128 * m:128 * (m + 1)],
                                     D2[0:D, sl], start=True, stop=False)
                    for gg in range(7):
                        nc.tensor.matmul(yp[:], w2s[:, gg, 128 * m:128 * (m + 1)],
                                         Zst[:, gg, sl],
                                         start=False, stop=(gg == 6))
                    nc.scalar.activation(y1[:, m, sl], yp[:], AF.Relu,
                                         bias=bt0s[:, m:m + 1])

                # -- top-MLP layer 2 --
                for m in range(2):
                    ps = mmp.tile([128, 512], F32)
                    for k in range(4):
                        nc.tensor.matmul(ps[:], wt1s[:, k, 128 * m:128 * (m + 1)],
                                         y1[:, k, sl], start=(k == 0), stop=(k == 3))
                    nc.scalar.activation(y2[:, m, sl], ps[:],
                                         AF.Relu, bias=bt1s[:, m:m + 1])

                # -- top-MLP layer 3 + sigmoid --
                ps = mmp.tile([128, 512], F32)
                for k in range(2):
                    nc.tensor.matmul(ps[0:1, :], wt2s[:, k, :],
                                     y2[:, k, sl], start=(k == 0), stop=(k == 1))
                nc.scalar.activation(outs[0:1, sl], ps[0:1, :],
                                     AF.Sigmoid, bias=bt2s[:, :])

            nc.sync.dma_start(out[:], outs[:])

    nc.finalize()
    return nc


_NC_CACHE = None


def _get_nc():
    global _NC_CACHE
    if _NC_CACHE is None:
        _NC_CACHE = _build_nc()
    return _NC_CACHE


def _rep_bias(b, parts=128):
    b = np.asarray(b, np.float32)
    if b.size < parts:
        assert parts % b.size == 0
        return np.tile(b, parts // b.size).reshape(parts, 1)
    return np.ascontiguousarray(b.reshape(-1, parts).T)


def _wrap16(x):
    """index list [B] -> ucode layout [128, B/16]: entry i at (i%16, i//16),
    replicated across the 8 Q7 core groups."""
    w = x.reshape(B // 16, 16).T
    return np.ascontiguousarray(np.tile(w, (8, 1)))


def _host_prep(dense, sparse_idx, emb, Wb0, bb0, Wb1, bb1, Wb2, bb2,
               Wt0, bt0, Wt1, bt1, Wt2, bt2):
    bf = ml_dtypes.bfloat16
    embg = np.ascontiguousarray(
        np.asarray(emb, np.float32).astype(bf).reshape(T, NU, 256))

    Wt0 = np.asarray(Wt0, np.float32)
    li, lj = np.tril_indices(NI, -1)
    W2full = np.zeros((NI, NI, T1), np.float32)
    W2full[li, lj] = 0.5 * Wt0[D:]
    W2full[lj, li] = 0.5 * Wt0[D:]
    # K-stacked + row-interleaved, ii-groups padded to 32 partitions:
    # w2[32*ii + r, g, :] = W2full[4g+ii, f(r), :]
    w2 = np.zeros((128, 7, T1), np.float32)
    rr = np.arange(27)
    fr = 4 * (rr % 7) + rr // 7          # f(r): row -> feature
    for ii in range(4):
        for g in range(7):
            i = 4 * g + ii
            if i < NI:
                w2[32 * ii + rr, g] = W2full[i, fr]
    w2 = np.ascontiguousarray(w2.reshape(128, 7 * T1).astype(bf))

    shared = dict(
        embg=embg,
        wb0=np.asarray(Wb0, np.float32).astype(bf),
        bb0=_rep_bias(bb0),
        wb1=np.asarray(Wb1, np.float32).astype(bf),
        bb1=_rep_bias(bb1),
        wb2=np.asarray(Wb2, np.float32).astype(bf),
        bb2=_rep_bias(bb2),
        wt0d=np.asarray(Wt0[:D], np.float32).astype(bf),
        w2st=w2,
        bt0=_rep_bias(bt0),
        wt1=np.asarray(Wt1, np.float32).astype(bf),
        bt1=_rep_bias(bt1),
        wt2=np.asarray(Wt2, np.float32).astype(bf),
        bt2=np.asarray(bt2, np.float32).reshape(1, 1),
    )

    dense = np.asarray(dense, np.float32)
    idx = np.asarray(sparse_idx).astype(np.int64)
    in_maps = []
    for core in range(N_CORES):
        sl = slice(core * B, (core + 1) * B)
        ishard = idx[sl]                          # [B, 26]
        gi = np.zeros((T, 128, B // 16), np.int16)
        gm = np.zeros((128, T, 3, 16), np.uint8)
        for t in range(T):
            it = ishard[:, t]
            gi[t] = _wrap16((it // 4).astype(np.int16))
            r = (it % 4).astype(np.int64)         # row within 512B unit
            rt = r.reshape(B // 128, 128).T       # [128p, 16blk]
            for rr_ in (1, 2, 3):
                gm[:, t, rr_ - 1, :] = (rt == rr_).astype(np.uint8)
        m = dict(shared)
        m["dense_t"] = np.ascontiguousarray(dense[sl].T.astype(bf))
        m["gidx"] = gi
        m["gmask"] = np.ascontiguousarray(gm)
        in_maps.append(m)
    return in_maps


def kernel(dense, sparse_idx, emb, Wb0, bb0, Wb1, bb1, Wb2, bb2,
           Wt0, bt0, Wt1, bt1, Wt2, bt2, _trace=False, _trace_kwargs=None):
    nc = _get_nc()
    in_maps = _host_prep(dense, sparse_idx, emb, Wb0, bb0, Wb1, bb1, Wb2, bb2,
                         Wt0, bt0, Wt1, bt1, Wt2, bt2)
    res = run_bass_kernel_spmd(nc, in_maps, core_ids=list(range(N_CORES)),
                               trace=_trace, **(_trace_kwargs or {}))
    outp = np.concatenate([res.results[c]["out"].reshape(-1)
                           for c in range(N_CORES)])
    if _trace:
        kernel._last_results = res
    return outp


# revision 17
# speedup vs baseline: 1.5872x; 1.5872x over previous
"""DLRM (26-table embedding + pairwise interaction + MLPs) on 8 Trainium2
NeuronCores, data-parallel over the batch (each core owns B/8 = 2048 samples
and a full replica of the embedding tables; no collectives needed).

v2 — pipelined around the SWDGE gather stream (the hard floor: ~3ns/descriptor
of Q7 emission, 53248 descriptors/core ~ 165us). All compute hides under it:

  1. bottom MLP 13->512->256->64 (lhsT=W matmuls), overlaps first gathers
  2. gathers chunked into 4 sample-groups x 26 tables (512 idx each) on 4
     SWDGE queues with a deep buffer pool, so descriptor emission never
     stalls on drains and the 4 queues drain concurrently
  3. per group: DVE predicated-merge (row-in-unit select), HWDGE
     DMA-transposes into feature-major slabs tmt[64h+d, rI, j] where rI is an
     interleaved feature order (feature 4g+ii at row 7ii+g) chosen so every
     later access pattern is a plain nested-stride walk
  4. grams: 2 samples per matmul via block-diagonal zero-padded rhs
     rz[128, j, 64] (cols 32h+r), lhsT = dense slab column; out [27, 64]
     holds Z of both pair samples side by side
  5. Z relayout (DVE, PSUM->SBUF) into K-stacked Zst[27*ii + r, g, s] so the
     interaction fold runs as 7 K=108 matmuls per output block instead of
     27 K=27 ones; W2 is symmetrized/half-weighted/zero-diag + row-permuted
     on the host to match
  6. top MLP 415->512->256->1 + sigmoid per 512-sample block, f32 output
"""

import sys

if "/opt/trn_rl_repo" not in sys.path:
    sys.path.insert(0, "/opt/trn_rl_repo")

import ml_dtypes
import numpy as np

import concourse.bass as bass
import concourse.mybir as mybir
import concourse.tile as tile
from concourse import bacc
from concourse.bass_utils import run_bass_kernel_spmd

F32 = mybir.dt.float32
BF16 = mybir.dt.bfloat16
I16 = mybir.dt.int16
U8 = mybir.dt.uint8
AF = mybir.ActivationFunctionType

N_CORES = 8
B_TOTAL = 16384
B = B_TOTAL // N_CORES  # 2048 samples per core
T = 26                  # embedding tables
V = 100000              # vocab per table
D = 64                  # embedding dim
NI = 27                 # interaction vectors per sample (26 tables + dense)
N_DENSE = 13
H1, H2 = 512, 256       # bottom MLP dims
T1, T2 = 512, 256       # top MLP dims
NU = V // 4             # 25000 gather units (512B = 4 vocab rows) per table
NG = 4                  # gather sample-groups (512 samples each)
NQ = 4                  # SWDGE queues


def _rpos(t):
    """feature t -> interleaved slab row 7*(t%4) + t//4 (t = 4g+ii -> 7ii+g)."""
    return 7 * (t % 4) + t // 4


def _build_nc():
    nc = bacc.Bacc(None, target_bir_lowering=False, num_swdge_queues=NQ)

    embg = nc.dram_tensor("embg", [T, NU, 256], BF16, kind="ExternalInput")
    gidx = nc.dram_tensor("gidx", [T, 128, B // 16], I16, kind="ExternalInput")
    gmask = nc.dram_tensor("gmask", [128, T, 3, 16], U8, kind="ExternalInput")
    dense_t = nc.dram_tensor("dense_t", [N_DENSE, B], BF16, kind="ExternalInput")
    wb0 = nc.dram_tensor("wb0", [N_DENSE, H1], BF16, kind="ExternalInput")
    bb0 = nc.dram_tensor("bb0", [128, H1 // 128], F32, kind="ExternalInput")
    wb1 = nc.dram_tensor("wb1", [H1, H2], BF16, kind="ExternalInput")
    bb1 = nc.dram_tensor("bb1", [128, H2 // 128], F32, kind="ExternalInput")
    wb2 = nc.dram_tensor("wb2", [H2, D], BF16, kind="ExternalInput")
    bb2 = nc.dram_tensor("bb2", [128, 1], F32, kind="ExternalInput")
    wt0d = nc.dram_tensor("wt0d", [D, T1], BF16, kind="ExternalInput")
    w2st = nc.dram_tensor("w2st", [128, 7 * T1], BF16, kind="ExternalInput")
    bt0 = nc.dram_tensor("bt0", [128, T1 // 128], F32, kind="ExternalInput")
    wt1 = nc.dram_tensor("wt1", [T1, T2], BF16, kind="ExternalInput")
    bt1 = nc.dram_tensor("bt1", [128, T2 // 128], F32, kind="ExternalInput")
    wt2 = nc.dram_tensor("wt2", [T2, 1], BF16, kind="ExternalInput")
    bt2 = nc.dram_tensor("bt2", [1, 1], F32, kind="ExternalInput")
    out = nc.dram_tensor("out", [1, B], F32, kind="ExternalOutput")

    with tile.TileContext(nc) as tc:
        with (
            tc.tile_pool(name="const", bufs=1) as cp,
            tc.tile_pool(name="mm", bufs=2, space="PSUM") as mmp,
            tc.tile_pool(name="gram", bufs=1, space="PSUM") as gramp,
            tc.tile_pool(name="fold", bufs=2, space="PSUM") as foldp,
            tc.tile_pool(name="tm", bufs=2) as tmp,
            tc.tile_pool(name="big", bufs=1) as bigp,
            tc.tile_pool(name="gth", bufs=8) as gthp,
            tc.tile_pool(name="fin", bufs=2) as finp,
        ):
            # ---- constants / weights ----
            wb0s = cp.tile([N_DENSE, H1], BF16)
            nc.sync.dma_start(wb0s[:], wb0[:])
            wb1s = cp.tile([128, 4, H2], BF16)
            for k in range(4):
                nc.sync.dma_start(wb1s[:, k, :], wb1[128 * k:128 * (k + 1), :])
            wb2s = cp.tile([128, 2, D], BF16)
            for k in range(2):
                nc.sync.dma_start(wb2s[:, k, :], wb2[128 * k:128 * (k + 1), :])
            wt0ds = cp.tile([D, T1], BF16)
            nc.sync.dma_start(wt0ds[:], wt0d[:])
            w2s = cp.tile([128, 7, T1], BF16)
            nc.sync.dma_start(w2s[:], w2st[:])
            wt1s = cp.tile([128, 4, T2], BF16)
            for k in range(4):
                nc.sync.dma_start(wt1s[:, k, :], wt1[128 * k:128 * (k + 1), :])
            wt2s = cp.tile([128, 2, 1], BF16)
            for k in range(2):
                nc.sync.dma_start(wt2s[:, k, :], wt2[128 * k:128 * (k + 1), :])
            bb0s = cp.tile([128, H1 // 128], F32)
            nc.sync.dma_start(bb0s[:], bb0[:])
            bb1s = cp.tile([128, H2 // 128], F32)
            nc.sync.dma_start(bb1s[:], bb1[:])
            bb2s = cp.tile([128, 1], F32)
            nc.sync.dma_start(bb2s[:], bb2[:])
            bt0s = cp.tile([128, T1 // 128], F32)
            nc.sync.dma_start(bt0s[:], bt0[:])
            bt1s = cp.tile([128, T2 // 128], F32)
            nc.sync.dma_start(bt1s[:], bt1[:])
            bt2s = cp.tile([1, 1], F32)
            nc.sync.dma_start(bt2s[:], bt2[:])
            dts = cp.tile([N_DENSE, B], BF16)
            nc.sync.dma_start(dts[:], dense_t[:])
            masks = cp.tile([128, T, 3, 16], U8)
            nc.sync.dma_start(masks[:], gmask[:])
            idxs = cp.tile([128, T, B // 16], I16)
            for t in range(T):
                nc.sync.dma_start(idxs[:, t, :], gidx[t, :, :])

            # d^T replicated on both partition halves: D2[64h + d, s]
            D2 = cp.tile([128, B], BF16)

            # zero-padded block-diagonal gram rhs, per local-pbt lb:
            # rz[64h'+d, lb, 32h + r, j] = slab value if h==h' else 0
            rz = bigp.tile([128, 2, 64, 128], BF16, tag="rz")
            nc.vector.memset(rz[:], 0)

            # K-stacked interaction matrix for the fold (ii-groups padded
            # to 32 partitions for the 32-aligned start rule; pad rows stay 0
            # and multiply zero w2st rows):
            # Zst[32*ii + r, g, s] = Z_s[f(r), 4g+ii]  (f = interleave inverse)
            Zst = bigp.tile([128, 7, B], BF16, tag="Zst")
            nc.vector.memset(Zst[:], 0)

            y1 = bigp.tile([128, 4, B], BF16, tag="y1")
            y2 = bigp.tile([128, 2, B], BF16, tag="y2")
            outs = bigp.tile([1, B], F32, tag="outs")

            # two persistent gram PSUM tiles, ping-ponged; memset once so the
            # 5 pad rows of each 32-band read back 0 in the full-width cast
            gps = [gramp.tile([128, 16, 64], F32, tag=f"gps{i}", name=f"gps{i}")
                   for i in range(2)]
            for i in range(2):
                nc.vector.memset(gps[i][:], 0)

            # ---- phase 1: bottom MLP (overlaps the first gathers) ----
            for n in range(4):
                sl = slice(512 * n, 512 * (n + 1))
                h1t = bigp.tile([128, 4, 512], BF16, tag=f"h1_{n % 2}")
                for m in range(4):
                    ps = mmp.tile([128, 512], F32)
                    nc.tensor.matmul(ps[:], wb0s[:, 128 * m:128 * (m + 1)],
                                     dts[:, sl], start=True, stop=True)
                    nc.scalar.activation(h1t[:, m, :], ps[:], AF.Relu,
                                         bias=bb0s[:, m:m + 1])
                h2t = bigp.tile([128, 2, 512], BF16, tag=f"h2_{n % 2}")
                for m in range(2):
                    ps = mmp.tile([128, 512], F32)
                    for k in range(4):
                        nc.tensor.matmul(ps[:], wb1s[:, k, 128 * m:128 * (m + 1)],
                                         h1t[:, k, :], start=(k == 0), stop=(k == 3))
                    nc.scalar.activation(h2t[:, m, :], ps[:], AF.Relu,
                                         bias=bb1s[:, m:m + 1])
                ps = mmp.tile([128, 512], F32)
                for half in range(2):   # write d to both partition halves
                    for k in range(2):
                        nc.tensor.matmul(ps[64 * half:64 * half + 64, :],
                                         wb2s[:, k, :], h2t[:, k, :],
                                         start=(k == 0), stop=(k == 1),
                                         tile_position=(0, 64 * half))
                nc.scalar.activation(D2[:, sl], ps[:], AF.Relu, bias=bb2s[:])

            # residue groups for 4-wide transposes: tables t = 4a + tq land on
            # interleaved slab rows 7tq + a; dense d is slab row _rpos(26)=20
            tqs = [[4 * a + tq for a in range(7 if tq < 2 else 6)]
                   for tq in range(4)]
            tqs[2] = tqs[2][:6]   # t=26 slot is filled from D2, not gathered

            # ---- phases 2-6, pipelined by 512-sample group ----
            gather_seq = 0
            for g in range(NG):
                # -- gather + merge this group's 512 samples for all tables --
                fin4s = []
                for tq in range(4):
                    na = len(tqs[tq])
                    fin4 = finp.tile([128, na, 4, D], BF16, tag=f"fin{tq}")
                    fin4s.append(fin4)
                    for a, t in enumerate(tqs[tq]):
                        gt = gthp.tile([128, 4, 256], BF16, tag="gt")
                        nc.gpsimd.dma_gather(gt[:], embg[t, :, :],
                                             idxs[:, t, 32 * g:32 * (g + 1)],
                                             512, 512, 256, single_packet=False,
                                             queue_num=gather_seq % NQ)
                        gather_seq += 1
                        for r in (1, 2, 3):
                            nc.vector.copy_predicated(
                                gt[:, :, 0:D],
                                masks[:, t, r - 1, 4 * g:4 * (g + 1)]
                                .to_broadcast([128, 4, D]),
                                gt[:, :, D * r:D * (r + 1)])
                        nc.scalar.activation(fin4[:, a, :, :], gt[:, :, 0:D],
                                             AF.Copy)

                # -- feature-major pair-slab via 4 grouped HWDGE transposes --
                tmt = tmp.tile([128, 28, 2, 128], BF16, tag="tmt")
                for tq in range(4):
                    na = len(tqs[tq])
                    eng = nc.sync if tq % 2 == 0 else nc.scalar
                    eng.dma_start_transpose(
                        tmt[:, 7 * tq:7 * tq + na, :, :],
                        fin4s[tq][:].rearrange("p a b d -> p (a b d)"))
                for lb in range(2):
                    for h in range(2):
                        s0 = 512 * g + 256 * lb + 128 * h
                        nc.vector.tensor_copy(
                            tmt[64 * h:64 * h + 64, _rpos(26), lb, :],
                            D2[64 * h:64 * h + 64, s0:s0 + 128])

                Zr = Zst[:].rearrange(
                    "p g (gg lb h sl c) -> p gg lb c sl h g",
                    gg=NG, lb=2, h=2, sl=32, c=4)
                for lb in range(2):
                    # -- zero-padded gram rhs via HWDGE sbuf-to-sbuf copies --
                    for h in range(2):
                        eng = nc.sync if h == 0 else nc.scalar
                        eng.dma_start(
                            rz[64 * h:64 * h + 64, lb, 32 * h:32 * h + 27, :],
                            tmt[64 * h:64 * h + 64, 0:27, lb, :])

                    # -- grams: 2 samples per matmul, j-pair (s, s+128) --
                    for half in range(2):
                        ps = gps[half]
                        for j in range(64 * half, 64 * half + 64):
                            c = j % 4
                            nc.tensor.matmul(
                                ps[32 * c:32 * c + 27, (j // 4) % 16, :],
                                tmt[:, 0:27, lb, j],
                                rz[:, lb, :, j],
                                start=True, stop=True,
                                tile_position=(0, 32 * c))
                        # -- stage 1: full-width PSUM -> SBUF bf16 cast --
                        sc = tmp.tile([128, 16, 64], BF16, tag="sc")
                        nc.scalar.activation(sc[:], ps[:], AF.Copy)
                        # -- stage 2: strided bf16 relayout into Zst --
                        scr = sc[:].rearrange("p s (h x) -> p s h x", h=2)
                        for c in range(4):
                            for ii in range(4):
                                nc.vector.tensor_copy(
                                    Zr[32 * ii:32 * ii + 27, g, lb, c,
                                       16 * half:16 * half + 16, :, :],
                                    scr[32 * c:32 * c + 27, :, :,
                                        7 * ii:7 * ii + 7])

                # -- fold + top-MLP layer 1 for this 512-sample block --
                sl = slice(512 * g, 512 * (g + 1))
                for m in range(4):
                    yp = foldp.tile([128, 512], F32)
                    nc.tensor.matmul(yp[:], wt0ds[:, 128 * m:128 * (m + 1)],
                                     D2[0:D, sl], start=True, stop=False)
                    for gg in range(7):
                        nc.tensor.matmul(yp[:], w2s[:, gg, 128 * m:128 * (m + 1)],
                                         Zst[:, gg, sl],
                                         start=False, stop=(gg == 6))
                    nc.scalar.activation(y1[:, m, sl], yp[:], AF.Relu,
                                         bias=bt0s[:, m:m + 1])

                # -- top-MLP layer 2 --
                for m in range(2):
                    ps = mmp.tile([128, 512], F32)
                    for k in range(4):
                        nc.tensor.matmul(ps[:], wt1s[:, k, 128 * m:128 * (m + 1)],
                                         y1[:, k, sl], start=(k == 0), stop=(k == 3))
                    nc.scalar.activation(y2[:, m, sl], ps[:],
                                         AF.Relu, bias=bt1s[:, m:m + 1])

                # -- top-MLP layer 3 + sigmoid --
                ps = mmp.tile([128, 512], F32)
                for k in range(2):
                    nc.tensor.matmul(ps[0:1, :], wt2s[:, k, :],
                                     y2[:, k, sl], start=(k == 0), stop=(k == 1))
                nc.scalar.activation(outs[0:1, sl], ps[0:1, :],
                                     AF.Sigmoid, bias=bt2s[:, :])

            nc.sync.dma_start(out[:], outs[:])

    nc.finalize()
    return nc


_NC_CACHE = None


def _get_nc():
    global _NC_CACHE
    if _NC_CACHE is None:
        _NC_CACHE = _build_nc()
    return _NC_CACHE


def _rep_bias(b, parts=128):
    b = np.asarray(b, np.float32)
    if b.size < parts:
        assert parts % b.size == 0
        return np.tile(b, parts // b.size).reshape(parts, 1)
    return np.ascontiguousarray(b.reshape(-1, parts).T)


def _wrap16(x):
    """index list [B] -> ucode layout [128, B/16]: entry i at (i%16, i//16),
    replicated across the 8 Q7 core groups."""
    w = x.reshape(B // 16, 16).T
    return np.ascontiguousarray(np.tile(w, (8, 1)))


def _host_prep(dense, sparse_idx, emb, Wb0, bb0, Wb1, bb1, Wb2, bb2,
               Wt0, bt0, Wt1, bt1, Wt2, bt2):
    bf = ml_dtypes.bfloat16
    embg = np.ascontiguousarray(
        np.asarray(emb, np.float32).astype(bf).reshape(T, NU, 256))

    Wt0 = np.asarray(Wt0, np.float32)
    li, lj = np.tril_indices(NI, -1)
    W2full = np.zeros((NI, NI, T1), np.float32)
    W2full[li, lj] = 0.5 * Wt0[D:]
    W2full[lj, li] = 0.5 * Wt0[D:]
    # K-stacked + row-interleaved, ii-groups padded to 32 partitions:
    # w2[32*ii + r, g, :] = W2full[4g+ii, f(r), :]
    w2 = np.zeros((128, 7, T1), np.float32)
    rr = np.arange(27)
    fr = 4 * (rr % 7) + rr // 7          # f(r): row -> feature
    for ii in range(4):
        for g in range(7):
            i = 4 * g + ii
            if i < NI:
                w2[32 * ii + rr, g] = W2full[i, fr]
    w2 = np.ascontiguousarray(w2.reshape(128, 7 * T1).astype(bf))

    shared = dict(
        embg=embg,
        wb0=np.asarray(Wb0, np.float32).astype(bf),
        bb0=_rep_bias(bb0),
        wb1=np.asarray(Wb1, np.float32).astype(bf),
        bb1=_rep_bias(bb1),
        wb2=np.asarray(Wb2, np.float32).astype(bf),
        bb2=_rep_bias(bb2),
        wt0d=np.asarray(Wt0[:D], np.float32).astype(bf),
        w2st=w2,
        bt0=_rep_bias(bt0),
        wt1=np.asarray(Wt1, np.float32).astype(bf),
        bt1=_rep_bias(bt1),
        wt2=np.asarray(Wt2, np.float32).astype(bf),
        bt2=np.asarray(bt2, np.float32).reshape(1, 1),
    )

    dense = np.asarray(dense, np.float32)
    idx = np.asarray(sparse_idx).astype(np.int64)
    in_maps = []
    for core in range(N_CORES):
        sl = slice(core * B, (core + 1) * B)
        ishard = idx[sl]                          # [B, 26]
        gi = np.zeros((T, 128, B // 16), np.int16)
        gm = np.zeros((128, T, 3, 16), np.uint8)
        for t in range(T):
            it = ishard[:, t]
            gi[t] = _wrap16((it // 4).astype(np.int16))
            r = (it % 4).astype(np.int64)         # row within 512B unit
            rt = r.reshape(B // 128, 128).T       # [128p, 16blk]
            for rr_ in (1, 2, 3):
                gm[:, t, rr_ - 1, :] = (rt == rr_).astype(np.uint8)
        m = dict(shared)
        m["dense_t"] = np.ascontiguousarray(dense[sl].T.astype(bf))
        m["gidx"] = gi
        m["gmask"] = np.ascontiguousarray(gm)
        in_maps.append(m)
    return in_maps


def kernel(dense, sparse_idx, emb, Wb0, bb0, Wb1, bb1, Wb2, bb2,
           Wt0, bt0, Wt1, bt1, Wt2, bt2, _trace=False, _trace_kwargs=None):
    nc = _get_nc()
    in_maps = _host_prep(dense, sparse_idx, emb, Wb0, bb0, Wb1, bb1, Wb2, bb2,
                         Wt0, bt0, Wt1, bt1, Wt2, bt2)
    res = run_bass_kernel_spmd(nc, in_maps, core_ids=list(range(N_CORES)),
                               trace=_trace, **(_trace_kwargs or {}))
    outp = np.concatenate([res.results[c]["out"].reshape(-1)
                           for c in range(N_CORES)])
    if _trace:
        kernel._last_results = res
    return outp


# revision 26
# speedup vs baseline: 1.8577x; 1.1704x over previous
"""DLRM (26-table embedding + pairwise interaction + MLPs) on 8 Trainium2
NeuronCores, data-parallel over the batch (each core owns B/8 = 2048 samples
and a full replica of the embedding tables; no collectives needed).

v2 — pipelined around the SWDGE gather stream (the hard floor: ~3ns/descriptor
of Q7 emission, 53248 descriptors/core ~ 165us). All compute hides under it:

  1. bottom MLP 13->512->256->64 (lhsT=W matmuls), overlaps first gathers
  2. gathers chunked into 4 sample-groups x 26 tables (512 idx each) on 4
     SWDGE queues with a deep buffer pool, so descriptor emission never
     stalls on drains and the 4 queues drain concurrently
  3. per group: DVE predicated-merge (row-in-unit select), HWDGE
     DMA-transposes into feature-major slabs tmt[64h+d, rI, j] where rI is an
     interleaved feature order (feature 4g+ii at row 7ii+g) chosen so every
     later access pattern is a plain nested-stride walk
  4. grams: 2 samples per matmul via block-diagonal zero-padded rhs
     rz[128, j, 64] (cols 32h+r), lhsT = dense slab column; out [27, 64]
     holds Z of both pair samples side by side
  5. Z relayout (DVE, PSUM->SBUF) into K-stacked Zst[27*ii + r, g, s] so the
     interaction fold runs as 7 K=108 matmuls per output block instead of
     27 K=27 ones; W2 is symmetrized/half-weighted/zero-diag + row-permuted
     on the host to match
  6. top MLP 415->512->256->1 + sigmoid per 512-sample block, f32 output
"""

import sys

if "/opt/trn_rl_repo" not in sys.path:
    sys.path.insert(0, "/opt/trn_rl_repo")

import ml_dtypes
import numpy as np

import concourse.bass as bass
import concourse.mybir as mybir
import concourse.tile as tile
from concourse import bacc
from concourse.bass_utils import run_bass_kernel_spmd

F32 = mybir.dt.float32
BF16 = mybir.dt.bfloat16
I16 = mybir.dt.int16
U8 = mybir.dt.uint8
AF = mybir.ActivationFunctionType

N_CORES = 8
B_TOTAL = 16384
B = B_TOTAL // N_CORES  # 2048 samples per core
T = 26                  # embedding tables
V = 100000              # vocab per table
D = 64                  # embedding dim
NI = 27                 # interaction vectors per sample (26 tables + dense)
N_DENSE = 13
H1, H2 = 512, 256       # bottom MLP dims
T1, T2 = 512, 256       # top MLP dims
NU = V // 4             # 25000 gather units (512B = 4 vocab rows) per table
NG = 4                  # gather sample-groups (512 samples each)
NQ = 4                  # SWDGE queues


def _rpos(t):
    """feature t -> interleaved slab row 7*(t%4) + t//4 (t = 4g+ii -> 7ii+g)."""
    return 7 * (t % 4) + t // 4


def _build_nc():
    nc = bacc.Bacc(None, target_bir_lowering=False, num_swdge_queues=NQ)

    embg = nc.dram_tensor("embg", [T, NU, 256], BF16, kind="ExternalInput")
    gidx = nc.dram_tensor("gidx", [T, 128, B // 16], I16, kind="ExternalInput")
    gmask = nc.dram_tensor("gmask", [128, T, 3, 16], U8, kind="ExternalInput")
    dense_t = nc.dram_tensor("dense_t", [N_DENSE, B], BF16, kind="ExternalInput")
    wb0 = nc.dram_tensor("wb0", [N_DENSE, H1], BF16, kind="ExternalInput")
    bb0 = nc.dram_tensor("bb0", [128, H1 // 128], F32, kind="ExternalInput")
    wb1 = nc.dram_tensor("wb1", [H1, H2], BF16, kind="ExternalInput")
    bb1 = nc.dram_tensor("bb1", [128, H2 // 128], F32, kind="ExternalInput")
    wb2 = nc.dram_tensor("wb2", [H2, D], BF16, kind="ExternalInput")
    bb2 = nc.dram_tensor("bb2", [128, 1], F32, kind="ExternalInput")
    wt0d = nc.dram_tensor("wt0d", [D, T1], BF16, kind="ExternalInput")
    w2st = nc.dram_tensor("w2st", [128, 7 * T1], BF16, kind="ExternalInput")
    bt0 = nc.dram_tensor("bt0", [128, T1 // 128], F32, kind="ExternalInput")
    wt1 = nc.dram_tensor("wt1", [T1, T2], BF16, kind="ExternalInput")
    bt1 = nc.dram_tensor("bt1", [128, T2 // 128], F32, kind="ExternalInput")
    wt2 = nc.dram_tensor("wt2", [T2, 1], BF16, kind="ExternalInput")
    bt2 = nc.dram_tensor("bt2", [1, 1], F32, kind="ExternalInput")
    out = nc.dram_tensor("out", [1, B], F32, kind="ExternalOutput")

    with tile.TileContext(nc) as tc:
        with (
            tc.tile_pool(name="const", bufs=1) as cp,
            tc.tile_pool(name="mm", bufs=2, space="PSUM") as mmp,
            tc.tile_pool(name="gram", bufs=1, space="PSUM") as gramp,
            tc.tile_pool(name="fold", bufs=2, space="PSUM") as foldp,
            tc.tile_pool(name="tm", bufs=2) as tmp,
            tc.tile_pool(name="big", bufs=1) as bigp,
            tc.tile_pool(name="gth", bufs=6) as gthp,
            tc.tile_pool(name="fin", bufs=2) as finp,
        ):
            # ---- constants / weights ----
            wb0s = cp.tile([N_DENSE, H1], BF16)
            nc.sync.dma_start(wb0s[:], wb0[:])
            wb1s = cp.tile([128, 4, H2], BF16)
            for k in range(4):
                nc.sync.dma_start(wb1s[:, k, :], wb1[128 * k:128 * (k + 1), :])
            wb2s = cp.tile([128, 2, D], BF16)
            for k in range(2):
                nc.sync.dma_start(wb2s[:, k, :], wb2[128 * k:128 * (k + 1), :])
            wt0ds = cp.tile([D, T1], BF16)
            nc.sync.dma_start(wt0ds[:], wt0d[:])
            w2s = cp.tile([128, 7, T1], BF16)
            nc.sync.dma_start(w2s[:], w2st[:])
            wt1s = cp.tile([128, 4, T2], BF16)
            for k in range(4):
                nc.sync.dma_start(wt1s[:, k, :], wt1[128 * k:128 * (k + 1), :])
            wt2s = cp.tile([128, 2, 1], BF16)
            for k in range(2):
                nc.sync.dma_start(wt2s[:, k, :], wt2[128 * k:128 * (k + 1), :])
            bb0s = cp.tile([128, H1 // 128], F32)
            nc.sync.dma_start(bb0s[:], bb0[:])
            bb1s = cp.tile([128, H2 // 128], F32)
            nc.sync.dma_start(bb1s[:], bb1[:])
            bb2s = cp.tile([128, 1], F32)
            nc.sync.dma_start(bb2s[:], bb2[:])
            bt0s = cp.tile([128, T1 // 128], F32)
            nc.sync.dma_start(bt0s[:], bt0[:])
            bt1s = cp.tile([128, T2 // 128], F32)
            nc.sync.dma_start(bt1s[:], bt1[:])
            bt2s = cp.tile([1, 1], F32)
            nc.sync.dma_start(bt2s[:], bt2[:])
            dts = cp.tile([N_DENSE, B], BF16)
            nc.sync.dma_start(dts[:], dense_t[:])
            masks = cp.tile([128, T, 3, 16], U8)
            nc.sync.dma_start(masks[:], gmask[:])
            idxs = cp.tile([128, T, B // 16], I16)
            for t in range(T):
                nc.sync.dma_start(idxs[:, t, :], gidx[t, :, :])

            # d^T replicated on both partition halves: D2[64h + d, s]
            D2 = cp.tile([128, B], BF16)

            # zero-padded block-diagonal gram rhs, per local-pbt lb:
            # rz[64h'+d, lb, 32h + r, j] = slab value if h==h' else 0
            rz = bigp.tile([128, 2, 64, 128], BF16, tag="rz")
            nc.vector.memset(rz[:], 0)

            # K-stacked interaction matrix for the fold (ii-groups padded
            # to 32 partitions for the 32-aligned start rule; pad rows stay 0
            # and multiply zero w2st rows). Free layout (s', g) with g
            # INNERMOST so the relayout moves aligned 7-element runs, and s'
            # a per-group permuted sample order
            #   s' = 512*G + 128c + 64lb + 32h + 16half + slot16
            # (sample s = 512*G + 256lb + 128h + 4*(16half+slot16) + c); the
            # permutation rides through fold/top-MLP and is undone at the end.
            # Zst[32*ii + r, s', g] = Z_s(s')[f(r), 4g+ii]
            Zst = bigp.tile([128, B, 7], BF16, tag="Zst")
            nc.vector.memset(Zst[:], 0)
            # stage-1 landing tile for the gram PSUM casts, (lb,half) major
            sc4 = bigp.tile([128, 4, 16, 64], BF16, tag="sc4")

            y1 = bigp.tile([128, 4, B], BF16, tag="y1")
            y2 = bigp.tile([128, 2, B], BF16, tag="y2")
            # natural-sample-order output; sigmoid writes land s'-permuted
            outn = bigp.tile([1, B], F32, tag="outn")
            onp = outn[:].rearrange("p (gg lb h hf st c) -> p gg c lb h hf st",
                                    gg=NG, lb=2, h=2, hf=2, st=16)

            # two persistent gram PSUM tiles, ping-ponged; memset once so the
            # 5 pad rows of each 32-band read back 0 in the full-width cast
            gps = [gramp.tile([128, 16, 64], F32, tag=f"gps{i}", name=f"gps{i}")
                   for i in range(2)]
            for i in range(2):
                nc.vector.memset(gps[i][:], 0)

            # ---- phase 1: bottom MLP (overlaps the first gathers) ----
            for n in range(4):
                sl = slice(512 * n, 512 * (n + 1))
                h1t = bigp.tile([128, 4, 512], BF16, tag="h1")
                for m in range(4):
                    ps = mmp.tile([128, 512], F32)
                    nc.tensor.matmul(ps[:], wb0s[:, 128 * m:128 * (m + 1)],
                                     dts[:, sl], start=True, stop=True)
                    nc.scalar.activation(h1t[:, m, :], ps[:], AF.Relu,
                                         bias=bb0s[:, m:m + 1])
                h2t = bigp.tile([128, 2, 512], BF16, tag="h2")
                for m in range(2):
                    ps = mmp.tile([128, 512], F32)
                    for k in range(4):
                        nc.tensor.matmul(ps[:], wb1s[:, k, 128 * m:128 * (m + 1)],
                                         h1t[:, k, :], start=(k == 0), stop=(k == 3))
                    nc.scalar.activation(h2t[:, m, :], ps[:], AF.Relu,
                                         bias=bb1s[:, m:m + 1])
                ps = mmp.tile([128, 512], F32)
                for half in range(2):   # write d to both partition halves
                    for k in range(2):
                        nc.tensor.matmul(ps[64 * half:64 * half + 64, :],
                                         wb2s[:, k, :], h2t[:, k, :],
                                         start=(k == 0), stop=(k == 1),
                                         tile_position=(0, 64 * half))
                nc.scalar.activation(D2[:, sl], ps[:], AF.Relu, bias=bb2s[:])

            # residue groups for 4-wide transposes: tables t = 4a + tq land on
            # interleaved slab rows 7tq + a; dense d is slab row _rpos(26)=20
            tqs = [[4 * a + tq for a in range(7 if tq < 2 else 6)]
                   for tq in range(4)]
            tqs[2] = tqs[2][:6]   # t=26 slot is filled from D2, not gathered

            # ---- phases 2-6, pipelined by 512-sample group ----
            gather_seq = 0
            for g in range(NG):
                # -- gather + merge this group's 512 samples for all tables --
                fin4s = []
                for tq in range(4):
                    na = len(tqs[tq])
                    fin4 = finp.tile([128, na, 4, D], BF16, tag=f"fin{tq}")
                    fin4s.append(fin4)
                    for a, t in enumerate(tqs[tq]):
                        gt = gthp.tile([128, 4, 256], BF16, tag="gt")
                        nc.gpsimd.dma_gather(gt[:], embg[t, :, :],
                                             idxs[:, t, 32 * g:32 * (g + 1)],
                                             512, 512, 256, single_packet=False,
                                             queue_num=gather_seq % NQ)
                        gather_seq += 1
                        for r in (1, 2, 3):
                            nc.vector.copy_predicated(
                                gt[:, :, 0:D],
                                masks[:, t, r - 1, 4 * g:4 * (g + 1)]
                                .to_broadcast([128, 4, D]),
                                gt[:, :, D * r:D * (r + 1)])
                        nc.scalar.activation(fin4[:, a, :, :], gt[:, :, 0:D],
                                             AF.Copy)

                # -- feature-major pair-slab via 4 grouped HWDGE transposes --
                tmt = tmp.tile([128, 28, 2, 128], BF16, tag="tmt")
                for tq in range(4):
                    na = len(tqs[tq])
                    eng = nc.sync if tq % 2 == 0 else nc.scalar
                    eng.dma_start_transpose(
                        tmt[:, 7 * tq:7 * tq + na, :, :],
                        fin4s[tq][:].rearrange("p a b d -> p (a b d)"))
                for lb in range(2):
                    for h in range(2):
                        s0 = 512 * g + 256 * lb + 128 * h
                        nc.vector.tensor_copy(
                            tmt[64 * h:64 * h + 64, _rpos(26), lb, :],
                            D2[64 * h:64 * h + 64, s0:s0 + 128])

                for lb in range(2):
                    # -- zero-padded gram rhs via HWDGE sbuf-to-sbuf copies --
                    for h in range(2):
                        eng = nc.sync if h == 0 else nc.scalar
                        eng.dma_start(
                            rz[64 * h:64 * h + 64, lb, 32 * h:32 * h + 27, :],
                            tmt[64 * h:64 * h + 64, 0:27, lb, :])

                    # -- grams: 2 samples per matmul, j-pair (s, s+128) --
                    for half in range(2):
                        ps = gps[half]
                        for j in range(64 * half, 64 * half + 64):
                            c = j % 4
                            nc.tensor.matmul(
                                ps[32 * c:32 * c + 27, (j // 4) % 16, :],
                                tmt[:, 0:27, lb, j],
                                rz[:, lb, :, j],
                                start=True, stop=True,
                                tile_position=(0, 32 * c))
                        # -- stage 1: full-width PSUM -> SBUF bf16 cast --
                        nc.scalar.activation(sc4[:, 2 * lb + half, :, :],
                                             ps[:], AF.Copy)
                    # -- stage 2: relayout sc4 -> Zst in aligned 7-elem runs --
                    # dims (h, half, slot16, g) on both sides
                    scr = sc4[:].rearrange(
                        "p (lb hf) sl (h x) -> p lb h hf sl x", lb=2, h=2)
                    Zr = Zst[:].rearrange(
                        "p (gg c lb h hf st) gdim -> p gg c lb h hf st gdim",
                        gg=NG, c=4, lb=2, h=2, hf=2)
                    for c in range(4):
                        for ii in range(4):
                            nc.vector.tensor_copy(
                                Zr[32 * ii:32 * ii + 27, g, c, lb],
                                scr[32 * c:32 * c + 27, lb, :, :, :,
                                    7 * ii:7 * ii + 7])

                # -- fold + top-MLP layer 1 for this 512-sample block --
                # columns are in s' order; the d-part rhs reads D2 through the
                # s'-permutation (per-c 4-dim strided APs)
                sl = slice(512 * g, 512 * (g + 1))
                D2p = D2[:].rearrange("p (gg lb h hf st c) -> p gg c lb h hf st",
                                      gg=NG, lb=2, h=2, hf=2, st=16)
                for m in range(4):
                    yp = foldp.tile([128, 512], F32)
                    for gg in range(7):
                        nc.tensor.matmul(yp[:], w2s[:, gg, 128 * m:128 * (m + 1)],
                                         Zst[:, sl, gg],
                                         start=(gg == 0), stop=False)
                    nc.tensor.matmul(yp[:], wt0ds[:, 128 * m:128 * (m + 1)],
                                     D2p[0:D, g], start=False, stop=True)
                    nc.scalar.activation(y1[:, m, sl], yp[:], AF.Relu,
                                         bias=bt0s[:, m:m + 1])

                # -- top-MLP layer 2 --
                for m in range(2):
                    ps = mmp.tile([128, 512], F32)
                    for k in range(4):
                        nc.tensor.matmul(ps[:], wt1s[:, k, 128 * m:128 * (m + 1)],
                                         y1[:, k, sl], start=(k == 0), stop=(k == 3))
                    nc.scalar.activation(y2[:, m, sl], ps[:],
                                         AF.Relu, bias=bt1s[:, m:m + 1])

                # -- top-MLP layer 3 + sigmoid (un-permuting on write) --
                ps = mmp.tile([128, 512], F32)
                for k in range(2):
                    nc.tensor.matmul(ps[0:1, :], wt2s[:, k, :],
                                     y2[:, k, sl], start=(k == 0), stop=(k == 1))
                for c in range(4):
                    nc.scalar.activation(
                        onp[0:1, g, c], ps[0:1, 128 * c:128 * (c + 1)]
                        .rearrange("p (lb h hf st) -> p lb h hf st",
                                   lb=2, h=2, hf=2),
                        AF.Sigmoid, bias=bt2s[:, :])

            nc.sync.dma_start(out[:], outn[:])

    nc.finalize()
    return nc


_NC_CACHE = None


def _get_nc():
    global _NC_CACHE
    if _NC_CACHE is None:
        _NC_CACHE = _build_nc()
    return _NC_CACHE


def _rep_bias(b, parts=128):
    b = np.asarray(b, np.float32)
    if b.size < parts:
        assert parts % b.size == 0
        return np.tile(b, parts // b.size).reshape(parts, 1)
    return np.ascontiguousarray(b.reshape(-1, parts).T)


def _wrap16(x):
    """index list [B] -> ucode layout [128, B/16]: entry i at (i%16, i//16),
    replicated across the 8 Q7 core groups."""
    w = x.reshape(B // 16, 16).T
    return np.ascontiguousarray(np.tile(w, (8, 1)))


def _host_prep(dense, sparse_idx, emb, Wb0, bb0, Wb1, bb1, Wb2, bb2,
               Wt0, bt0, Wt1, bt1, Wt2, bt2):
    bf = ml_dtypes.bfloat16
    embg = np.ascontiguousarray(
        np.asarray(emb, np.float32).astype(bf).reshape(T, NU, 256))

    Wt0 = np.asarray(Wt0, np.float32)
    li, lj = np.tril_indices(NI, -1)
    W2full = np.zeros((NI, NI, T1), np.float32)
    W2full[li, lj] = 0.5 * Wt0[D:]
    W2full[lj, li] = 0.5 * Wt0[D:]
    # K-stacked + row-interleaved, ii-groups padded to 32 partitions:
    # w2[32*ii + r, g, :] = W2full[4g+ii, f(r), :]
    w2 = np.zeros((128, 7, T1), np.float32)
    rr = np.arange(27)
    fr = 4 * (rr % 7) + rr // 7          # f(r): row -> feature
    for ii in range(4):
        for g in range(7):
            i = 4 * g + ii
            if i < NI:
                w2[32 * ii + rr, g] = W2full[i, fr]
    w2 = np.ascontiguousarray(w2.reshape(128, 7 * T1).astype(bf))

    shared = dict(
        embg=embg,
        wb0=np.asarray(Wb0, np.float32).astype(bf),
        bb0=_rep_bias(bb0),
        wb1=np.asarray(Wb1, np.float32).astype(bf),
        bb1=_rep_bias(bb1),
        wb2=np.asarray(Wb2, np.float32).astype(bf),
        bb2=_rep_bias(bb2),
        wt0d=np.asarray(Wt0[:D], np.float32).astype(bf),
        w2st=w2,
        bt0=_rep_bias(bt0),
        wt1=np.asarray(Wt1, np.float32).astype(bf),
        bt1=_rep_bias(bt1),
        wt2=np.asarray(Wt2, np.float32).astype(bf),
        bt2=np.asarray(bt2, np.float32).reshape(1, 1),
    )

    dense = np.asarray(dense, np.float32)
    idx = np.asarray(sparse_idx).astype(np.int64)
    in_maps = []
    for core in range(N_CORES):
        sl = slice(core * B, (core + 1) * B)
        ishard = idx[sl]                          # [B, 26]
        gi = np.zeros((T, 128, B // 16), np.int16)
        gm = np.zeros((128, T, 3, 16), np.uint8)
        for t in range(T):
            it = ishard[:, t]
            gi[t] = _wrap16((it // 4).astype(np.int16))
            r = (it % 4).astype(np.int64)         # row within 512B unit
            rt = r.reshape(B // 128, 128).T       # [128p, 16blk]
            for rr_ in (1, 2, 3):
                gm[:, t, rr_ - 1, :] = (rt == rr_).astype(np.uint8)
        m = dict(shared)
        m["dense_t"] = np.ascontiguousarray(dense[sl].T.astype(bf))
        m["gidx"] = gi
        m["gmask"] = np.ascontiguousarray(gm)
        in_maps.append(m)
    return in_maps


def kernel(dense, sparse_idx, emb, Wb0, bb0, Wb1, bb1, Wb2, bb2,
           Wt0, bt0, Wt1, bt1, Wt2, bt2, _trace=False, _trace_kwargs=None):
    nc = _get_nc()
    in_maps = _host_prep(dense, sparse_idx, emb, Wb0, bb0, Wb1, bb1, Wb2, bb2,
                         Wt0, bt0, Wt1, bt1, Wt2, bt2)
    res = run_bass_kernel_spmd(nc, in_maps, core_ids=list(range(N_CORES)),
                               trace=_trace, **(_trace_kwargs or {}))
    outp = np.concatenate([res.results[c]["out"].reshape(-1)
                           for c in range(N_CORES)])
    if _trace:
        kernel._last_results = res
    return outp


# revision 27
# speedup vs baseline: 1.9931x; 1.0729x over previous
"""DLRM (26-table embedding + pairwise interaction + MLPs) on 8 Trainium2
NeuronCores, data-parallel over the batch (each core owns B/8 = 2048 samples
and a full replica of the embedding tables; no collectives needed).

v2 — pipelined around the SWDGE gather stream (the hard floor: ~3ns/descriptor
of Q7 emission, 53248 descriptors/core ~ 165us). All compute hides under it:

  1. bottom MLP 13->512->256->64 (lhsT=W matmuls), overlaps first gathers
  2. gathers chunked into 4 sample-groups x 26 tables (512 idx each) on 4
     SWDGE queues with a deep buffer pool, so descriptor emission never
     stalls on drains and the 4 queues drain concurrently
  3. per group: DVE predicated-merge (row-in-unit select), HWDGE
     DMA-transposes into feature-major slabs tmt[64h+d, rI, j] where rI is an
     interleaved feature order (feature 4g+ii at row 7ii+g) chosen so every
     later access pattern is a plain nested-stride walk
  4. grams: 2 samples per matmul via block-diagonal zero-padded rhs
     rz[128, j, 64] (cols 32h+r), lhsT = dense slab column; out [27, 64]
     holds Z of both pair samples side by side
  5. Z relayout (DVE, PSUM->SBUF) into K-stacked Zst[27*ii + r, g, s] so the
     interaction fold runs as 7 K=108 matmuls per output block instead of
     27 K=27 ones; W2 is symmetrized/half-weighted/zero-diag + row-permuted
     on the host to match
  6. top MLP 415->512->256->1 + sigmoid per 512-sample block, f32 output
"""

import sys

if "/opt/trn_rl_repo" not in sys.path:
    sys.path.insert(0, "/opt/trn_rl_repo")

import ml_dtypes
import numpy as np

import concourse.bass as bass
import concourse.mybir as mybir
import concourse.tile as tile
from concourse import bacc
from concourse.bass_utils import run_bass_kernel_spmd

F32 = mybir.dt.float32
BF16 = mybir.dt.bfloat16
I16 = mybir.dt.int16
U8 = mybir.dt.uint8
AF = mybir.ActivationFunctionType

N_CORES = 8
B_TOTAL = 16384
B = B_TOTAL // N_CORES  # 2048 samples per core
T = 26                  # embedding tables
V = 100000              # vocab per table
D = 64                  # embedding dim
NI = 27                 # interaction vectors per sample (26 tables + dense)
N_DENSE = 13
H1, H2 = 512, 256       # bottom MLP dims
T1, T2 = 512, 256       # top MLP dims
NU = V // 4             # 25000 gather units (512B = 4 vocab rows) per table
NG = 4                  # gather sample-groups (512 samples each)
NQ = 4                  # SWDGE queues


def _rpos(t):
    """feature t -> interleaved slab row 7*(t%4) + t//4 (t = 4g+ii -> 7ii+g)."""
    return 7 * (t % 4) + t // 4


def _build_nc():
    nc = bacc.Bacc(None, target_bir_lowering=False, num_swdge_queues=NQ)

    embg = nc.dram_tensor("embg", [T, NU, 256], BF16, kind="ExternalInput")
    gidx = nc.dram_tensor("gidx", [T, 128, B // 16], I16, kind="ExternalInput")
    gmask = nc.dram_tensor("gmask", [128, T, 3, 16], U8, kind="ExternalInput")
    dense_t = nc.dram_tensor("dense_t", [N_DENSE, B], BF16, kind="ExternalInput")
    wb0 = nc.dram_tensor("wb0", [N_DENSE, H1], BF16, kind="ExternalInput")
    bb0 = nc.dram_tensor("bb0", [128, H1 // 128], F32, kind="ExternalInput")
    wb1 = nc.dram_tensor("wb1", [H1, H2], BF16, kind="ExternalInput")
    bb1 = nc.dram_tensor("bb1", [128, H2 // 128], F32, kind="ExternalInput")
    wb2 = nc.dram_tensor("wb2", [H2, D], BF16, kind="ExternalInput")
    bb2 = nc.dram_tensor("bb2", [128, 1], F32, kind="ExternalInput")
    wt0d = nc.dram_tensor("wt0d", [D, T1], BF16, kind="ExternalInput")
    w2st = nc.dram_tensor("w2st", [128, 7 * T1], BF16, kind="ExternalInput")
    bt0 = nc.dram_tensor("bt0", [128, T1 // 128], F32, kind="ExternalInput")
    wt1 = nc.dram_tensor("wt1", [T1, T2], BF16, kind="ExternalInput")
    bt1 = nc.dram_tensor("bt1", [128, T2 // 128], F32, kind="ExternalInput")
    wt2 = nc.dram_tensor("wt2", [T2, 1], BF16, kind="ExternalInput")
    bt2 = nc.dram_tensor("bt2", [1, 1], F32, kind="ExternalInput")
    out = nc.dram_tensor("out", [1, B], F32, kind="ExternalOutput")

    with tile.TileContext(nc) as tc:
        with (
            tc.tile_pool(name="const", bufs=1) as cp,
            tc.tile_pool(name="mm", bufs=2, space="PSUM") as mmp,
            tc.tile_pool(name="gram", bufs=1, space="PSUM") as gramp,
            tc.tile_pool(name="fold", bufs=2, space="PSUM") as foldp,
            tc.tile_pool(name="tm", bufs=2) as tmp,
            tc.tile_pool(name="big", bufs=1) as bigp,
            tc.tile_pool(name="gth", bufs=8) as gthp,
            tc.tile_pool(name="fin", bufs=2) as finp,
        ):
            # ---- gather indices first: the gather stream is the critical
            # path and must start before the weight loads queue up ----
            idxs = cp.tile([128, T, B // 16], I16)
            for t in range(T):
                nc.sync.dma_start(idxs[:, t, :], gidx[t, :, :])
            masks = cp.tile([128, T, 3, 16], U8)
            nc.sync.dma_start(masks[:], gmask[:])

            # ---- constants / weights ----
            wb0s = cp.tile([N_DENSE, H1], BF16)
            nc.sync.dma_start(wb0s[:], wb0[:])
            wb1s = cp.tile([128, 4, H2], BF16)
            for k in range(4):
                nc.sync.dma_start(wb1s[:, k, :], wb1[128 * k:128 * (k + 1), :])
            wb2s = cp.tile([128, 2, D], BF16)
            for k in range(2):
                nc.sync.dma_start(wb2s[:, k, :], wb2[128 * k:128 * (k + 1), :])
            wt0ds = cp.tile([D, T1], BF16)
            nc.sync.dma_start(wt0ds[:], wt0d[:])
            w2s = cp.tile([128, 7, T1], BF16)
            nc.sync.dma_start(w2s[:], w2st[:])
            wt1s = cp.tile([128, 4, T2], BF16)
            for k in range(4):
                nc.sync.dma_start(wt1s[:, k, :], wt1[128 * k:128 * (k + 1), :])
            wt2s = cp.tile([128, 2, 1], BF16)
            for k in range(2):
                nc.sync.dma_start(wt2s[:, k, :], wt2[128 * k:128 * (k + 1), :])
            bb0s = cp.tile([128, H1 // 128], F32)
            nc.sync.dma_start(bb0s[:], bb0[:])
            bb1s = cp.tile([128, H2 // 128], F32)
            nc.sync.dma_start(bb1s[:], bb1[:])
            bb2s = cp.tile([128, 1], F32)
            nc.sync.dma_start(bb2s[:], bb2[:])
            bt0s = cp.tile([128, T1 // 128], F32)
            nc.sync.dma_start(bt0s[:], bt0[:])
            bt1s = cp.tile([128, T2 // 128], F32)
            nc.sync.dma_start(bt1s[:], bt1[:])
            bt2s = cp.tile([1, 1], F32)
            nc.sync.dma_start(bt2s[:], bt2[:])
            dts = cp.tile([N_DENSE, B], BF16)
            nc.sync.dma_start(dts[:], dense_t[:])

            # d^T replicated on both partition halves: D2[64h + d, s]
            D2 = cp.tile([128, B], BF16)

            # zero-padded block-diagonal gram rhs, per local-pbt lb:
            # rz[64h'+d, lb, 32h + r, j] = slab value if h==h' else 0
            rz = bigp.tile([128, 2, 64, 128], BF16, tag="rz")
            nc.vector.memset(rz[:], 0)

            # K-stacked interaction matrix for the fold (ii-groups padded
            # to 32 partitions for the 32-aligned start rule; pad rows stay 0
            # and multiply zero w2st rows). Free layout (s', g) with g
            # INNERMOST so the relayout moves aligned 7-element runs, and s'
            # a per-group permuted sample order
            #   s' = 512*G + 128c + 64lb + 32h + 16half + slot16
            # (sample s = 512*G + 256lb + 128h + 4*(16half+slot16) + c); the
            # permutation rides through fold/top-MLP and is undone at the end.
            # Zst[32*ii + r, s', g] = Z_s(s')[f(r), 4g+ii]
            Zst = bigp.tile([128, B, 7], BF16, tag="Zst")
            nc.vector.memset(Zst[:], 0)
            # stage-1 landing tile for the gram PSUM casts, (lb,half) major
            sc4 = bigp.tile([128, 4, 16, 64], BF16, tag="sc4")

            y1 = bigp.tile([128, 4, B], BF16, tag="y1")
            y2 = bigp.tile([128, 2, B], BF16, tag="y2")
            # natural-sample-order output; sigmoid writes land s'-permuted
            outn = bigp.tile([1, B], F32, tag="outn")
            onp = outn[:].rearrange("p (gg lb h hf st c) -> p gg c lb h hf st",
                                    gg=NG, lb=2, h=2, hf=2, st=16)

            # two persistent gram PSUM tiles, ping-ponged; memset once so the
            # 5 pad rows of each 32-band read back 0 in the full-width cast
            gps = [gramp.tile([128, 16, 64], F32, tag=f"gps{i}", name=f"gps{i}")
                   for i in range(2)]
            for i in range(2):
                nc.vector.memset(gps[i][:], 0)

            # ---- phase 1: bottom MLP (overlaps the first gathers) ----
            for n in range(4):
                sl = slice(512 * n, 512 * (n + 1))
                h1t = bigp.tile([128, 4, 512], BF16, tag="h1")
                for m in range(4):
                    ps = mmp.tile([128, 512], F32)
                    nc.tensor.matmul(ps[:], wb0s[:, 128 * m:128 * (m + 1)],
                                     dts[:, sl], start=True, stop=True)
                    nc.scalar.activation(h1t[:, m, :], ps[:], AF.Relu,
                                         bias=bb0s[:, m:m + 1])
                h2t = bigp.tile([128, 2, 512], BF16, tag="h2")
                for m in range(2):
                    ps = mmp.tile([128, 512], F32)
                    for k in range(4):
                        nc.tensor.matmul(ps[:], wb1s[:, k, 128 * m:128 * (m + 1)],
                                         h1t[:, k, :], start=(k == 0), stop=(k == 3))
                    nc.scalar.activation(h2t[:, m, :], ps[:], AF.Relu,
                                         bias=bb1s[:, m:m + 1])
                ps = mmp.tile([128, 512], F32)
                for half in range(2):   # write d to both partition halves
                    for k in range(2):
                        nc.tensor.matmul(ps[64 * half:64 * half + 64, :],
                                         wb2s[:, k, :], h2t[:, k, :],
                                         start=(k == 0), stop=(k == 1),
                                         tile_position=(0, 64 * half))
                nc.scalar.activation(D2[:, sl], ps[:], AF.Relu, bias=bb2s[:])

            # residue groups for 4-wide transposes: tables t = 4a + tq land on
            # interleaved slab rows 7tq + a; dense d is slab row _rpos(26)=20
            tqs = [[4 * a + tq for a in range(7 if tq < 2 else 6)]
                   for tq in range(4)]
            tqs[2] = tqs[2][:6]   # t=26 slot is filled from D2, not gathered

            # ---- phases 2-6, pipelined by 512-sample group ----
            gather_seq = 0
            for g in range(NG):
                # -- gather + merge this group's 512 samples for all tables --
                fin4s = []
                for tq in range(4):
                    na = len(tqs[tq])
                    fin4 = finp.tile([128, na, 4, D], BF16, tag=f"fin{tq}")
                    fin4s.append(fin4)
                    for a, t in enumerate(tqs[tq]):
                        gt = gthp.tile([128, 4, 256], BF16, tag="gt")
                        nc.gpsimd.dma_gather(gt[:], embg[t, :, :],
                                             idxs[:, t, 32 * g:32 * (g + 1)],
                                             512, 512, 256, single_packet=False,
                                             queue_num=gather_seq % NQ)
                        gather_seq += 1
                        for r in (1, 2, 3):
                            nc.vector.copy_predicated(
                                gt[:, :, 0:D],
                                masks[:, t, r - 1, 4 * g:4 * (g + 1)]
                                .to_broadcast([128, 4, D]),
                                gt[:, :, D * r:D * (r + 1)])
                        nc.scalar.activation(fin4[:, a, :, :], gt[:, :, 0:D],
                                             AF.Copy)

                # -- feature-major pair-slab via 4 grouped HWDGE transposes --
                tmt = tmp.tile([128, 28, 2, 128], BF16, tag="tmt")
                for tq in range(4):
                    na = len(tqs[tq])
                    eng = nc.sync if tq % 2 == 0 else nc.scalar
                    eng.dma_start_transpose(
                        tmt[:, 7 * tq:7 * tq + na, :, :],
                        fin4s[tq][:].rearrange("p a b d -> p (a b d)"))
                for lb in range(2):
                    for h in range(2):
                        s0 = 512 * g + 256 * lb + 128 * h
                        nc.vector.tensor_copy(
                            tmt[64 * h:64 * h + 64, _rpos(26), lb, :],
                            D2[64 * h:64 * h + 64, s0:s0 + 128])

                for lb in range(2):
                    # -- zero-padded gram rhs via HWDGE sbuf-to-sbuf copies --
                    for h in range(2):
                        eng = nc.sync if h == 0 else nc.scalar
                        eng.dma_start(
                            rz[64 * h:64 * h + 64, lb, 32 * h:32 * h + 27, :],
                            tmt[64 * h:64 * h + 64, 0:27, lb, :])

                    # -- grams: 2 samples per matmul, j-pair (s, s+128) --
                    for half in range(2):
                        ps = gps[half]
                        for j in range(64 * half, 64 * half + 64):
                            c = j % 4
                            nc.tensor.matmul(
                                ps[32 * c:32 * c + 27, (j // 4) % 16, :],
                                tmt[:, 0:27, lb, j],
                                rz[:, lb, :, j],
                                start=True, stop=True,
                                tile_position=(0, 32 * c))
                        # -- stage 1: full-width PSUM -> SBUF bf16 cast --
                        nc.scalar.activation(sc4[:, 2 * lb + half, :, :],
                                             ps[:], AF.Copy)
                    # -- stage 2: relayout sc4 -> Zst in aligned 7-elem runs --
                    # dims (h, half, slot16, g) on both sides
                    scr = sc4[:].rearrange(
                        "p (lb hf) sl (h x) -> p lb h hf sl x", lb=2, h=2)
                    Zr = Zst[:].rearrange(
                        "p (gg c lb h hf st) gdim -> p gg c lb h hf st gdim",
                        gg=NG, c=4, lb=2, h=2, hf=2)
                    for c in range(4):
                        for ii in range(4):
                            nc.vector.tensor_copy(
                                Zr[32 * ii:32 * ii + 27, g, c, lb],
                                scr[32 * c:32 * c + 27, lb, :, :, :,
                                    7 * ii:7 * ii + 7])

                # -- fold + top-MLP layer 1 for this 512-sample block --
                # columns are in s' order; the d-part rhs reads D2 through the
                # s'-permutation (per-c 4-dim strided APs)
                sl = slice(512 * g, 512 * (g + 1))
                D2p = D2[:].rearrange("p (gg lb h hf st c) -> p gg c lb h hf st",
                                      gg=NG, lb=2, h=2, hf=2, st=16)
                for m in range(4):
                    yp = foldp.tile([128, 512], F32)
                    for gg in range(7):
                        nc.tensor.matmul(yp[:], w2s[:, gg, 128 * m:128 * (m + 1)],
                                         Zst[:, sl, gg],
                                         start=(gg == 0), stop=False)
                    nc.tensor.matmul(yp[:], wt0ds[:, 128 * m:128 * (m + 1)],
                                     D2p[0:D, g], start=False, stop=True)
                    nc.scalar.activation(y1[:, m, sl], yp[:], AF.Relu,
                                         bias=bt0s[:, m:m + 1])

                # -- top-MLP layer 2 --
                for m in range(2):
                    ps = mmp.tile([128, 512], F32)
                    for k in range(4):
                        nc.tensor.matmul(ps[:], wt1s[:, k, 128 * m:128 * (m + 1)],
                                         y1[:, k, sl], start=(k == 0), stop=(k == 3))
                    nc.scalar.activation(y2[:, m, sl], ps[:],
                                         AF.Relu, bias=bt1s[:, m:m + 1])

                # -- top-MLP layer 3 + sigmoid (un-permuting on write) --
                ps = mmp.tile([128, 512], F32)
                for k in range(2):
                    nc.tensor.matmul(ps[0:1, :], wt2s[:, k, :],
                                     y2[:, k, sl], start=(k == 0), stop=(k == 1))
                for c in range(4):
                    nc.scalar.activation(
                        onp[0:1, g, c], ps[0:1, 128 * c:128 * (c + 1)]
                        .rearrange("p (lb h hf st) -> p lb h hf st",
                                   lb=2, h=2, hf=2),
                        AF.Sigmoid, bias=bt2s[:, :])

            nc.sync.dma_start(out[:], outn[:])

    nc.finalize()
    return nc


_NC_CACHE = None


def _get_nc():
    global _NC_CACHE
    if _NC_CACHE is None:
        _NC_CACHE = _build_nc()
    return _NC_CACHE


def _rep_bias(b, parts=128):
    b = np.asarray(b, np.float32)
    if b.size < parts:
        assert parts % b.size == 0
        return np.tile(b, parts // b.size).reshape(parts, 1)
    return np.ascontiguousarray(b.reshape(-1, parts).T)


def _wrap16(x):
    """index list [B] -> ucode layout [128, B/16]: entry i at (i%16, i//16),
    replicated across the 8 Q7 core groups."""
    w = x.reshape(B // 16, 16).T
    return np.ascontiguousarray(np.tile(w, (8, 1)))


def _host_prep(dense, sparse_idx, emb, Wb0, bb0, Wb1, bb1, Wb2, bb2,
               Wt0, bt0, Wt1, bt1, Wt2, bt2):
    bf = ml_dtypes.bfloat16
    embg = np.ascontiguousarray(
        np.asarray(emb, np.float32).astype(bf).reshape(T, NU, 256))

    Wt0 = np.asarray(Wt0, np.float32)
    li, lj = np.tril_indices(NI, -1)
    W2full = np.zeros((NI, NI, T1), np.float32)
    W2full[li, lj] = 0.5 * Wt0[D:]
    W2full[lj, li] = 0.5 * Wt0[D:]
    # K-stacked + row-interleaved, ii-groups padded to 32 partitions:
    # w2[32*ii + r, g, :] = W2full[4g+ii, f(r), :]
    w2 = np.zeros((128, 7, T1), np.float32)
    rr = np.arange(27)
    fr = 4 * (rr % 7) + rr // 7          # f(r): row -> feature
    for ii in range(4):
        for g in range(7):
            i = 4 * g + ii
            if i < NI:
                w2[32 * ii + rr, g] = W2full[i, fr]
    w2 = np.ascontiguousarray(w2.reshape(128, 7 * T1).astype(bf))

    shared = dict(
        embg=embg,
        wb0=np.asarray(Wb0, np.float32).astype(bf),
        bb0=_rep_bias(bb0),
        wb1=np.asarray(Wb1, np.float32).astype(bf),
        bb1=_rep_bias(bb1),
        wb2=np.asarray(Wb2, np.float32).astype(bf),
        bb2=_rep_bias(bb2),
        wt0d=np.asarray(Wt0[:D], np.float32).astype(bf),
        w2st=w2,
        bt0=_rep_bias(bt0),
        wt1=np.asarray(Wt1, np.float32).astype(bf),
        bt1=_rep_bias(bt1),
        wt2=np.asarray(Wt2, np.float32).astype(bf),
        bt2=np.asarray(bt2, np.float32).reshape(1, 1),
    )

    dense = np.asarray(dense, np.float32)
    idx = np.asarray(sparse_idx).astype(np.int64)
    in_maps = []
    for core in range(N_CORES):
        sl = slice(core * B, (core + 1) * B)
        ishard = idx[sl]                          # [B, 26]
        gi = np.zeros((T, 128, B // 16), np.int16)
        gm = np.zeros((128, T, 3, 16), np.uint8)
        for t in range(T):
            it = ishard[:, t]
            gi[t] = _wrap16((it // 4).astype(np.int16))
            r = (it % 4).astype(np.int64)         # row within 512B unit
            rt = r.reshape(B // 128, 128).T       # [128p, 16blk]
            for rr_ in (1, 2, 3):
                gm[:, t, rr_ - 1, :] = (rt == rr_).astype(np.uint8)
        m = dict(shared)
        m["dense_t"] = np.ascontiguousarray(dense[sl].T.astype(bf))
        m["gidx"] = gi
        m["gmask"] = np.ascontiguousarray(gm)
        in_maps.append(m)
    return in_maps


def kernel(dense, sparse_idx, emb, Wb0, bb0, Wb1, bb1, Wb2, bb2,
           Wt0, bt0, Wt1, bt1, Wt2, bt2, _trace=False, _trace_kwargs=None):
    nc = _get_nc()
    in_maps = _host_prep(dense, sparse_idx, emb, Wb0, bb0, Wb1, bb1, Wb2, bb2,
                         Wt0, bt0, Wt1, bt1, Wt2, bt2)
    res = run_bass_kernel_spmd(nc, in_maps, core_ids=list(range(N_CORES)),
                               trace=_trace, **(_trace_kwargs or {}))
    outp = np.concatenate([res.results[c]["out"].reshape(-1)
                           for c in range(N_CORES)])
    if _trace:
        kernel._last_results = res
    return outp
